# revision 1
# baseline (speedup 1.0000x reference)
"""Trainium2 Bass kernel for a dense transformer block (B=64,T=256,C=1024,H=16).

Sharding: pure data-parallel over batch across 8 NeuronCores (8 sequences
per core, no collectives). Per-core program:
  phase A (per batch): LN1 -> h(bf16) -> PE-transpose -> hT -> QKV (bf16
  matmuls, fp32 accum) -> causal attention in transposed layout
  (scoresT[s,t] so softmax sum + attn@V need no per-head transposes) ->
  proj -> residual -> spill x2 to DRAM.
  phase B (per 256-token supertile): LN2 -> y -> transpose -> yT(bf16) ->
  FFN (z1T = relu(w1.T y) interleaved with z2 accumulation) -> LN3 ->
  final residual -> out.
"""

from contextlib import ExitStack

import ml_dtypes
import numpy as np

import concourse.bass as bass
import concourse.bacc as bacc
import concourse.mybir as mybir
import concourse.tile as tile
from concourse.bass_utils import run_bass_kernel_spmd

F32 = mybir.dt.float32
F32R = mybir.dt.float32r
BF16 = mybir.dt.bfloat16
AF = mybir.ActivationFunctionType
ALU = mybir.AluOpType
AX = mybir.AxisListType

B, T, C, H, D = 64, 256, 1024, 16, 64
NCORES = 8
NB = B // NCORES          # 8 sequences per core
TOK = NB * T              # 2048 tokens per core
F4 = 4 * C                # 4096
EPS = 1e-3
SCALE = C ** -0.5         # 1/32

_CACHE = {}


def _ln_tile(nc, pools, xt, out_t, g_t, be_t, ncols=C):
    """LayerNorm of one [128, ncols] fp32 tile along the free axis.
    var computed as E[x^2] - mean^2. out_t may equal xt (in-place)."""
    stat, scr = pools["stat"], pools["scr"]
    s = stat.tile([128, 1], F32, tag="st", name="st")
    ss = stat.tile([128, 1], F32, tag="st", name="st")
    m = stat.tile([128, 1], F32, tag="st", name="st")
    v = stat.tile([128, 1], F32, tag="st", name="st")
    r = stat.tile([128, 1], F32, tag="st", name="st")
    mr = stat.tile([128, 1], F32, tag="st", name="st")
    sq = scr.tile([128, ncols], F32, tag="sq", name="sq")
    nc.vector.reduce_sum(s[:], xt[:], axis=AX.X)
    nc.scalar.activation(sq[:], xt[:], AF.Square, accum_out=ss[:])
    nc.vector.tensor_scalar_mul(m[:], s[:], 1.0 / ncols)
    # v = ss/ncols - m*m
    nc.vector.tensor_scalar_mul(v[:], ss[:], 1.0 / ncols)
    nc.vector.tensor_tensor(mr[:], m[:], m[:], ALU.mult)
    nc.vector.tensor_tensor(v[:], v[:], mr[:], ALU.subtract)
    nc.vector.tensor_scalar_add(v[:], v[:], EPS)
    nc.scalar.activation(r[:], v[:], AF.Sqrt)
    nc.vector.reciprocal(r[:], r[:])
    nc.vector.tensor_tensor(mr[:], m[:], r[:], ALU.mult)
    # out = (x*r - m*r) * g + be
    nc.vector.tensor_scalar(out_t[:], xt[:], r[:], mr[:], ALU.mult, ALU.subtract)
    nc.vector.tensor_tensor(out_t[:], out_t[:], g_t[:], ALU.mult)
    nc.vector.tensor_tensor(out_t[:], out_t[:], be_t[:], ALU.add)


def _build():
    nc = bacc.Bacc(target_bir_lowering=False)
    x_d = nc.dram_tensor("x", [TOK, C], F32, kind="ExternalInput")
    wq_d = nc.dram_tensor("wqf", [C, C], BF16, kind="ExternalInput")
    wk_d = nc.dram_tensor("wkf", [C, C], BF16, kind="ExternalInput")
    wv_d = nc.dram_tensor("wvf", [C, C], BF16, kind="ExternalInput")
    wp_d = nc.dram_tensor("wpf", [C, C], BF16, kind="ExternalInput")
    w1_d = nc.dram_tensor("w1f", [C, F4], BF16, kind="ExternalInput")
    w2_d = nc.dram_tensor("w2f", [F4, C], BF16, kind="ExternalInput")
    b1_d = nc.dram_tensor("b1t", [128, F4 // 128], F32, kind="ExternalInput")
    consts_bf = {}
    for nm in ["bprojb", "b2b", "g1b", "be1b", "g2b", "be2b", "g3b", "be3b"]:
        consts_bf[nm] = nc.dram_tensor(nm, [128, C], BF16, kind="ExternalInput")
    m0_d = nc.dram_tensor("mask0", [128, 128], BF16, kind="ExternalInput")
    m1_d = nc.dram_tensor("mask1", [128, 256], BF16, kind="ExternalInput")
    id_d = nc.dram_tensor("identb", [128, 128], BF16, kind="ExternalInput")
    ones_d = nc.dram_tensor("onesb", [128, 1], BF16, kind="ExternalInput")
    onesr_d = nc.dram_tensor("onesr", [1, 64], F32R, kind="ExternalInput")
    out_d = nc.dram_tensor("out", [TOK, C], F32, kind="ExternalOutput")
    x2_d = nc.dram_tensor("x2d", [TOK, C], F32)

    with tile.TileContext(nc) as tc, ExitStack() as ctx:
        const = ctx.enter_context(tc.tile_pool(name="const", bufs=1))
        cb_t = {nm: const.tile([128, C], BF16, tag=nm, name=nm)
                for nm in consts_bf}
        for nm, t in cb_t.items():
            nc.sync.dma_start(out=t[:], in_=consts_bf[nm][:, :])
        m0 = const.tile([128, 128], BF16, tag="m0", name="m0")
        nc.sync.dma_start(out=m0[:], in_=m0_d[:, :])
        m1 = const.tile([128, 256], BF16, tag="m1", name="m1")
        nc.sync.dma_start(out=m1[:], in_=m1_d[:, :])
        idb = const.tile([128, 128], BF16, tag="idb", name="idb")
        nc.sync.dma_start(out=idb[:], in_=id_d[:, :])
        ones = const.tile([128, 1], BF16, tag="ones", name="ones")
        nc.sync.dma_start(out=ones[:], in_=ones_d[:, :])
        onesr = const.tile([1, 64], F32R, tag="onesr", name="onesr")
        nc.sync.dma_start(out=onesr[:], in_=onesr_d[:, :])
        b1t = const.tile([128, F4 // 128], F32, tag="b1t", name="b1t")
        nc.sync.dma_start(out=b1t[:], in_=b1_d[:, :])

        stat = ctx.enter_context(tc.tile_pool(name="stat", bufs=24))
        scr = ctx.enter_context(tc.tile_pool(name="scr", bufs=2))
        pools = {"stat": stat, "scr": scr}

        # ---------------- phase A: attention ----------------
        with ExitStack() as actx:
            wpool = actx.enter_context(tc.tile_pool(name="wqkv", bufs=1))
            wq_sb, wk_sb, wv_sb, wp_sb = [], [], [], []
            for cb in range(8):
                for lst, dram, nm in ((wq_sb, wq_d, "wq"), (wk_sb, wk_d, "wk"),
                                      (wv_sb, wv_d, "wv"), (wp_sb, wp_d, "wp")):
                    t = wpool.tile([128, C], BF16, tag=f"{nm}{cb}", name=f"{nm}{cb}")
                    nc.sync.dma_start(out=t[:], in_=dram[cb * 128:(cb + 1) * 128, :])
                    lst.append(t)

            xb_p = actx.enter_context(tc.tile_pool(name="xb", bufs=4))
            h_p = actx.enter_context(tc.tile_pool(name="h", bufs=4))
            ht_p = actx.enter_context(tc.tile_pool(name="ht", bufs=10))
            qt_p = actx.enter_context(tc.tile_pool(name="qt", bufs=10))
            kt_p = actx.enter_context(tc.tile_pool(name="kt", bufs=10))
            v_p = actx.enter_context(tc.tile_pool(name="v", bufs=4))
            ex_p = actx.enter_context(tc.tile_pool(name="ex", bufs=6))
            cat_p = actx.enter_context(tc.tile_pool(name="cat", bufs=10))
            x2_p = actx.enter_context(tc.tile_pool(name="x2", bufs=4))
            rb_p = actx.enter_context(tc.tile_pool(name="rb", bufs=4))
            rec_p = actx.enter_context(tc.tile_pool(name="rec", bufs=4))
            ps = actx.enter_context(tc.tile_pool(name="psA", bufs=8, space="PSUM"))

            for b in range(NB):
                xb = [xb_p.tile([128, C], F32, tag="xb", name="xb") for _ in range(2)]
                hbf = [h_p.tile([128, C], BF16, tag="h", name="h") for _ in range(2)]
                for tb in range(2):
                    row = b * T + tb * 128
                    nc.sync.dma_start(out=xb[tb][:], in_=x_d[row:row + 128, :])
                    _ln_tile(nc, pools, xb[tb], hbf[tb], cb_t["g1b"], cb_t["be1b"])
                # transpose h -> hT (8 tiles [128c, 256t] bf16)
                ht = []
                for cb in range(8):
                    pt = ps.tile([128, 256], BF16, tag="ps", name="ps")
                    for tb in range(2):
                        nc.tensor.transpose(
                            pt[:, tb * 128:(tb + 1) * 128],
                            hbf[tb][:, cb * 128:(cb + 1) * 128], idb[:])
                    t = ht_p.tile([128, 256], BF16, tag="ht", name="ht")
                    nc.vector.tensor_copy(t[:], pt[:])
                    ht.append(t)
                # QKV
                qt, kt = [], []
                for p in range(8):
                    pq = ps.tile([128, 256], F32, tag="ps", name="ps")
                    pk = ps.tile([128, 256], F32, tag="ps", name="ps")
                    for cb in range(8):
                        st, sp = (cb == 0), (cb == 7)
                        nc.tensor.matmul(pq[:], wq_sb[cb][:, p * 128:(p + 1) * 128],
                                         ht[cb][:], start=st, stop=sp)
                        nc.tensor.matmul(pk[:], wk_sb[cb][:, p * 128:(p + 1) * 128],
                                         ht[cb][:], start=st, stop=sp)
                    tq = qt_p.tile([128, 256], BF16, tag="qt", name="qt")
                    tk = kt_p.tile([128, 256], BF16, tag="kt", name="kt")
                    nc.vector.tensor_copy(tq[:], pq[:])
                    nc.vector.tensor_copy(tk[:], pk[:])
                    qt.append(tq)
                    kt.append(tk)
                vsb = []
                for sb in range(2):
                    pv = [ps.tile([128, 512], F32, tag="ps", name="ps") for _ in range(2)]
                    for cb in range(8):
                        st, sp = (cb == 0), (cb == 7)
                        for n in range(2):
                            nc.tensor.matmul(
                                pv[n][:], ht[cb][:, sb * 128:(sb + 1) * 128],
                                wv_sb[cb][:, n * 512:(n + 1) * 512],
                                start=st, stop=sp)
                    tv = v_p.tile([128, C], BF16, tag="v", name="v")
                    for n in range(2):
                        nc.vector.tensor_copy(tv[:, n * 512:(n + 1) * 512], pv[n][:])
                    vsb.append(tv)
                # attention per head
                cat = [cat_p.tile([128, 256], BF16, tag="cat", name="cat") for _ in range(8)]
                for hh in range(H):
                    pr, off = hh // 2, (hh % 2) * 64
                    qs = qt[pr][off:off + 64, :]
                    ks = kt[pr][off:off + 64, :]
                    sc0 = ps.tile([128, 256], F32, tag="ps", name="ps")
                    sc1 = ps.tile([128, 256], F32, tag="ps", name="ps")
                    nc.tensor.matmul(sc0[:], ks[:, 0:128], qs[:])
                    nc.tensor.matmul(sc1[:], ks[:, 128:256], qs[:])
                    e0 = ex_p.tile([128, 256], BF16, tag="ex", name="ex")
                    e1 = ex_p.tile([128, 256], BF16, tag="ex", name="ex")
                    nc.scalar.activation(e0[:], sc0[:], AF.Exp, scale=SCALE)
                    nc.scalar.activation(e1[:], sc1[:], AF.Exp, scale=SCALE)
                    nc.vector.tensor_tensor(e0[:, 0:128], e0[:, 0:128], m0[:], ALU.mult)
                    nc.vector.tensor_tensor(e1[:], e1[:], m1[:], ALU.mult)
                    psum = ps.tile([1, 256], F32, tag="ps", name="ps")
                    nc.tensor.matmul(psum[:], ones[:], e0[:], start=True, stop=False)
                    nc.tensor.matmul(psum[:], ones[:], e1[:], start=False, stop=True)
                    rec = rec_p.tile([1, 256], F32R, tag="rec", name="rec")
                    with nc.allow_low_precision(reason="f32r recip for bcast mm"):
                        nc.vector.reciprocal(rec[:], psum[:])
                    prb = ps.tile([64, 256], F32, tag="ps", name="ps")
                    nc.tensor.matmul(prb[:], onesr[:], rec[:])
                    rb = rb_p.tile([64, 256], F32, tag="rb", name="rb")
                    nc.vector.tensor_copy(rb[:], prb[:])
                    po = ps.tile([64, 256], F32, tag="ps", name="ps")
                    nc.tensor.matmul(po[:], vsb[0][:, hh * 64:(hh + 1) * 64], e0[:],
                                     start=True, stop=False)
                    nc.tensor.matmul(po[:], vsb[1][:, hh * 64:(hh + 1) * 64], e1[:],
                                     start=False, stop=True)
                    nc.vector.tensor_tensor(cat[pr][off:off + 64, :], po[:], rb[:],
                                            ALU.mult)
                # proj + residual -> x2 -> DRAM spill
                for tb in range(2):
                    x2t = x2_p.tile([128, C], F32, tag="x2", name="x2")
                    for n in range(2):
                        pp = ps.tile([128, 512], F32, tag="ps", name="ps")
                        for cb in range(8):
                            nc.tensor.matmul(
                                pp[:], cat[cb][:, tb * 128:(tb + 1) * 128],
                                wp_sb[cb][:, n * 512:(n + 1) * 512],
                                start=(cb == 0), stop=(cb == 7))
                        nsl = slice(n * 512, (n + 1) * 512)
                        nc.vector.tensor_tensor(x2t[:, nsl], pp[:], xb[tb][:, nsl],
                                                ALU.add)
                        nc.vector.tensor_tensor(x2t[:, nsl], x2t[:, nsl],
                                                cb_t["bprojb"][:, nsl], ALU.add)
                    row = b * T + tb * 128
                    nc.sync.dma_start(out=x2_d[row:row + 128, :], in_=x2t[:])

        # ---------------- phase B: FFN ----------------
        with ExitStack() as bctx:
            w1_sb, w2_sb = [], []
            wpoolB = bctx.enter_context(tc.tile_pool(name="wffn", bufs=1))
            for cb in range(8):
                t = wpoolB.tile([128, F4], BF16, tag=f"w1_{cb}", name=f"w1_{cb}")
                nc.sync.dma_start(out=t[:], in_=w1_d[cb * 128:(cb + 1) * 128, :])
                w1_sb.append(t)
            for fb in range(32):
                t = wpoolB.tile([128, C], BF16, tag=f"w2_{fb}", name=f"w2_{fb}")
                nc.sync.dma_start(out=t[:], in_=w2_d[fb * 128:(fb + 1) * 128, :])
                w2_sb.append(t)

            x2B_p = bctx.enter_context(tc.tile_pool(name="x2B", bufs=3))
            y_p = bctx.enter_context(tc.tile_pool(name="y", bufs=3))
            ybf_p = bctx.enter_context(tc.tile_pool(name="ybf", bufs=2))
            yt_p = bctx.enter_context(tc.tile_pool(name="yt", bufs=8))
            z1_p = bctx.enter_context(tc.tile_pool(name="z1", bufs=3))
            u_p = bctx.enter_context(tc.tile_pool(name="u", bufs=2))
            psB = bctx.enter_context(tc.tile_pool(name="psB", bufs=8, space="PSUM"))

            for stx in range(NB):
                x2t = [x2B_p.tile([128, C], F32, tag="x2B", name="x2B") for _ in range(2)]
                yt_ = [y_p.tile([128, C], F32, tag="y", name="y") for _ in range(2)]
                ybf = [ybf_p.tile([128, C], BF16, tag="ybf", name="ybf") for _ in range(2)]
                for tb in range(2):
                    row = stx * 256 + tb * 128
                    nc.sync.dma_start(out=x2t[tb][:], in_=x2_d[row:row + 128, :])
                    _ln_tile(nc, pools, x2t[tb], yt_[tb], cb_t["g2b"], cb_t["be2b"])
                    nc.vector.tensor_copy(ybf[tb][:], yt_[tb][:])
                ytT = []
                for cb in range(8):
                    pt = psB.tile([128, 256], BF16, tag="ps", name="ps")
                    for tb in range(2):
                        nc.tensor.transpose(
                            pt[:, tb * 128:(tb + 1) * 128],
                            ybf[tb][:, cb * 128:(cb + 1) * 128], idb[:])
                    t = yt_p.tile([128, 256], BF16, tag="yt", name="yt")
                    nc.vector.tensor_copy(t[:], pt[:])
                    ytT.append(t)
                z2ps = [psB.tile([128, 512], F32, tag="ps", name="ps") for _ in range(4)]
                for fb in range(32):
                    pz = psB.tile([128, 256], F32, tag="ps", name="ps")
                    for cb in range(8):
                        nc.tensor.matmul(pz[:],
                                         w1_sb[cb][:, fb * 128:(fb + 1) * 128],
                                         ytT[cb][:], start=(cb == 0), stop=(cb == 7))
                    z1 = z1_p.tile([128, 256], BF16, tag="z1", name="z1")
                    nc.scalar.activation(z1[:], pz[:], AF.Relu,
                                         bias=b1t[:, fb:fb + 1])
                    for tb in range(2):
                        for n in range(2):
                            nc.tensor.matmul(
                                z2ps[tb * 2 + n][:],
                                z1[:, tb * 128:(tb + 1) * 128],
                                w2_sb[fb][:, n * 512:(n + 1) * 512],
                                start=(fb == 0), stop=(fb == 31))
                for tb in range(2):
                    u = u_p.tile([128, C], F32, tag="u", name="u")
                    for n in range(2):
                        nsl = slice(n * 512, (n + 1) * 512)
                        nc.vector.tensor_tensor(u[:, nsl], z2ps[tb * 2 + n][:],
                                                yt_[tb][:, nsl], ALU.add)
                    nc.vector.tensor_tensor(u[:], u[:], cb_t["b2b"][:], ALU.add)
                    _ln_tile(nc, pools, u, u, cb_t["g3b"], cb_t["be3b"])
                    nc.vector.tensor_tensor(x2t[tb][:], x2t[tb][:], u[:], ALU.add)
                    row = stx * 256 + tb * 128
                    nc.sync.dma_start(out=out_d[row:row + 128, :], in_=x2t[tb][:])
    nc.finalize()
    return nc


def _get_nc():
    if "nc" not in _CACHE:
        _CACHE["nc"] = _build()
    return _CACHE["nc"]


def kernel(x, wq, wk, wv, w_proj, b_proj, w1, b1, w2, b2,
           g1, be1, g2, be2, g3, be3):
    nc = _get_nc()
    bf = ml_dtypes.bfloat16
    x = np.asarray(x, np.float32)

    def bc(vec):
        return np.ascontiguousarray(
            np.broadcast_to(np.asarray(vec, np.float32).reshape(1, C),
                            (128, C))).astype(bf)

    wqf = np.ascontiguousarray(
        np.asarray(wq, np.float32).transpose(1, 0, 2).reshape(C, C)).astype(bf)
    wkf = np.ascontiguousarray(
        np.asarray(wk, np.float32).transpose(1, 0, 2).reshape(C, C)).astype(bf)
    wvf = np.ascontiguousarray(
        np.asarray(wv, np.float32).transpose(1, 0, 2).reshape(C, C)).astype(bf)
    wpf = np.asarray(w_proj, np.float32).astype(bf)
    w1f = np.asarray(w1, np.float32).astype(bf)
    w2f = np.asarray(w2, np.float32).astype(bf)
    b1t = np.ascontiguousarray(
        np.asarray(b1, np.float32).reshape(F4 // 128, 128).T)
    s = np.arange(128)[:, None]
    t = np.arange(128)[None, :]
    m0 = (s <= t).astype(np.float32).astype(bf)
    m1 = np.concatenate([np.zeros((128, 128), np.float32),
                         (s <= t).astype(np.float32)], axis=1).astype(bf)
    common = {
        "wqf": wqf, "wkf": wkf, "wvf": wvf, "wpf": wpf,
        "w1f": w1f, "w2f": w2f, "b1t": b1t,
        "bprojb": bc(b_proj), "b2b": bc(b2),
        "g1b": bc(g1), "be1b": bc(be1), "g2b": bc(g2), "be2b": bc(be2),
        "g3b": bc(g3), "be3b": bc(be3),
        "mask0": m0, "mask1": m1,
        "identb": np.eye(128, dtype=np.float32).astype(bf),
        "onesb": np.ones((128, 1), np.float32).astype(bf),
        "onesr": np.ones((1, 64), np.float32),
    }
    xs = x.reshape(NCORES, TOK, C)
    in_maps = [dict(common, x=np.ascontiguousarray(xs[i]))
               for i in range(NCORES)]
    import os
    trace = bool(os.environ.get("KERNEL_TRACE"))
    res = run_bass_kernel_spmd(nc, in_maps, core_ids=list(range(NCORES)),
                               trace=trace)
    _CACHE["last_res"] = res
    out = np.stack([res.results[i]["out"] for i in range(NCORES)], axis=0)
    return out.reshape(B, T, C).astype(np.float32)



# revision 7
# speedup vs baseline: 1.3437x; 1.3437x over previous
"""Trainium2 Bass kernel for a dense transformer block (B=64,T=256,C=1024,H=16).

Sharding: pure data-parallel over batch across 8 NeuronCores (8 sequences
per core, no collectives). Per-core program:
  phase A (per batch): LN1 (bn_stats) -> h(bf16) -> PE-transpose -> hT ->
  QKV (bf16 matmuls, fp32 accum) -> causal attention with scoresT[s,t]
  tiles; softmax denominator via N=1 matmuls vs ones, attn@V emits
  out[t,d] so the normalization is a per-partition scalar multiply on
  the Scalar engine -> PE-transpose cat back to [c,t] -> proj ->
  residual -> spill x2 to DRAM.
  phase B (per 256-token supertile): LN2 -> yT(bf16) -> FFN (z1T =
  relu(w1.T y) interleaved with z2 accumulation) -> LN3 -> final
  residual -> out.
"""

from contextlib import ExitStack

import ml_dtypes
import numpy as np

import concourse.bass as bass
import concourse.bacc as bacc
import concourse.mybir as mybir
import concourse.tile as tile
from concourse.bass_utils import run_bass_kernel_spmd

F32 = mybir.dt.float32
BF16 = mybir.dt.bfloat16
AF = mybir.ActivationFunctionType
ALU = mybir.AluOpType
AX = mybir.AxisListType

B, T, C, H, D = 64, 256, 1024, 16, 64
NCORES = 8
NB = B // NCORES          # 8 sequences per core
TOK = NB * T              # 2048 tokens per core
F4 = 4 * C                # 4096
EPS = 1e-3
SCALE = C ** -0.5         # 1/32

_CACHE = {}


def _ln_tile(nc, pools, xt, out_t, affine, ncols=C):
    """LayerNorm of one [128, ncols] tile along the free axis via bn_stats.
    out_t may equal xt (in-place). affine = (g_t, be_t) or None."""
    stat = pools["stat"]
    nsub = ncols // 512
    st = stat.tile([128, nsub, 6], F32, tag="bst", name="bst")
    mv = stat.tile([128, 2], F32, tag="bmv", name="bmv")
    rs = stat.tile([128, 1], F32, tag="brs", name="brs")
    xv = xt[:].rearrange("p (a b) -> p a b", b=512)
    for i in range(nsub):
        nc.vector.bn_stats(st[:, i, :], xv[:, i, :])
    nc.vector.bn_aggr(mv[:], st[:])
    nc.scalar.activation(rs[:], mv[:, 1:2], AF.Sqrt, bias=pools["eps"][:])
    nc.vector.reciprocal(rs[:], rs[:])
    nc.vector.tensor_scalar(out_t[:], xt[:], mv[:, 0:1], rs[:],
                            ALU.subtract, ALU.mult)
    if affine is not None:
        g_t, be_t = affine
        nc.vector.tensor_tensor(out_t[:], out_t[:], g_t[:], ALU.mult)
        nc.vector.tensor_tensor(out_t[:], out_t[:], be_t[:], ALU.add)


def _build(flags):
    aff1, aff2, aff3, use_bproj, use_b1, use_b2 = flags
    nc = bacc.Bacc(target_bir_lowering=False)
    x_d = nc.dram_tensor("x", [TOK, C], F32, kind="ExternalInput")
    wq_d = nc.dram_tensor("wqf", [C, C], BF16, kind="ExternalInput")
    wk_d = nc.dram_tensor("wkf", [C, C], BF16, kind="ExternalInput")
    wv_d = nc.dram_tensor("wvf", [C, C], BF16, kind="ExternalInput")
    wp_d = nc.dram_tensor("wpf", [C, C], BF16, kind="ExternalInput")
    w1_d = nc.dram_tensor("w1f", [C, F4], BF16, kind="ExternalInput")
    w2_d = nc.dram_tensor("w2f", [F4, C], BF16, kind="ExternalInput")
    consts_bf = {}
    names = []
    if use_b1:
        b1_d = nc.dram_tensor("b1t", [128, F4 // 128], F32, kind="ExternalInput")
    if use_bproj:
        names.append("bprojb")
    if use_b2:
        names.append("b2b")
    if aff1:
        names += ["g1b", "be1b"]
    if aff2:
        names += ["g2b", "be2b"]
    if aff3:
        names += ["g3b", "be3b"]
    for nm in names:
        consts_bf[nm] = nc.dram_tensor(nm, [128, C], BF16, kind="ExternalInput")
    m0_d = nc.dram_tensor("mask0", [128, 128], BF16, kind="ExternalInput")
    id_d = nc.dram_tensor("identb", [128, 128], BF16, kind="ExternalInput")
    ones_d = nc.dram_tensor("onesb", [128, 1], BF16, kind="ExternalInput")
    out_d = nc.dram_tensor("out", [TOK, C], F32, kind="ExternalOutput")
    x2_d = nc.dram_tensor("x2d", [TOK, C], F32)

    with tile.TileContext(nc) as tc, ExitStack() as ctx:
        const = ctx.enter_context(tc.tile_pool(name="const", bufs=1))
        cb_t = {nm: const.tile([128, C], BF16, tag=nm, name=nm)
                for nm in consts_bf}
        for nm, t in cb_t.items():
            nc.sync.dma_start(out=t[:], in_=consts_bf[nm][:, :])
        m0 = const.tile([128, 128], BF16, tag="m0", name="m0")
        nc.sync.dma_start(out=m0[:], in_=m0_d[:, :])
        idb = const.tile([128, 128], BF16, tag="idb", name="idb")
        nc.sync.dma_start(out=idb[:], in_=id_d[:, :])
        ones = const.tile([128, 1], BF16, tag="ones", name="ones")
        nc.sync.dma_start(out=ones[:], in_=ones_d[:, :])
        if use_b1:
            b1t = const.tile([128, F4 // 128], F32, tag="b1t", name="b1t")
            nc.sync.dma_start(out=b1t[:], in_=b1_d[:, :])

        epsb = const.tile([128, 1], F32, tag="eps", name="eps")
        nc.gpsimd.memset(epsb[:], EPS)

        stat = ctx.enter_context(tc.tile_pool(name="stat", bufs=8))
        pools = {"stat": stat, "eps": epsb}

        # ---------------- phase A: attention ----------------
        with ExitStack() as actx:
            wpool = actx.enter_context(tc.tile_pool(name="wqkv", bufs=1))
            wq_sb, wk_sb, wv_sb, wp_sb = [], [], [], []
            for cb in range(8):
                for lst, dram, nm in ((wq_sb, wq_d, "wq"), (wk_sb, wk_d, "wk"),
                                      (wv_sb, wv_d, "wv"), (wp_sb, wp_d, "wp")):
                    t = wpool.tile([128, C], BF16, tag=f"{nm}{cb}", name=f"{nm}{cb}")
                    nc.sync.dma_start(out=t[:], in_=dram[cb * 128:(cb + 1) * 128, :])
                    lst.append(t)

            xb_p = actx.enter_context(tc.tile_pool(name="xb", bufs=4))
            h_p = actx.enter_context(tc.tile_pool(name="h", bufs=4))
            ht_p = actx.enter_context(tc.tile_pool(name="ht", bufs=12))
            qt_p = actx.enter_context(tc.tile_pool(name="qt", bufs=12))
            kt_p = actx.enter_context(tc.tile_pool(name="kt", bufs=12))
            v_p = actx.enter_context(tc.tile_pool(name="v", bufs=4))
            ex_p = actx.enter_context(tc.tile_pool(name="ex", bufs=8))
            cat_p = actx.enter_context(tc.tile_pool(name="cat", bufs=4))
            ctt_p = actx.enter_context(tc.tile_pool(name="ctt", bufs=10))
            rec_p = actx.enter_context(tc.tile_pool(name="rec", bufs=12))
            x2_p = actx.enter_context(tc.tile_pool(name="x2", bufs=4))
            ps = actx.enter_context(tc.tile_pool(name="psA", bufs=8, space="PSUM"))

            for b in range(NB):
                xb = [xb_p.tile([128, C], F32, tag="xb", name="xb") for _ in range(2)]
                hbf = [h_p.tile([128, C], BF16, tag="h", name="h") for _ in range(2)]
                for tb in range(2):
                    row = b * T + tb * 128
                    nc.sync.dma_start(out=xb[tb][:], in_=x_d[row:row + 128, :])
                    _ln_tile(nc, pools, xb[tb], hbf[tb],
                             (cb_t["g1b"], cb_t["be1b"]) if aff1 else None)
                # transpose h -> hT (8 tiles [128c, 256t] bf16)
                ht = []
                for cb in range(8):
                    pt = ps.tile([128, 256], BF16, tag="ps", name="ps")
                    for tb in range(2):
                        nc.tensor.transpose(
                            pt[:, tb * 128:(tb + 1) * 128],
                            hbf[tb][:, cb * 128:(cb + 1) * 128], idb[:])
                    t = ht_p.tile([128, 256], BF16, tag="ht", name="ht")
                    if cb % 2 == 0:
                        nc.vector.tensor_copy(t[:], pt[:])
                    else:
                        nc.scalar.copy(t[:], pt[:])
                    ht.append(t)
                # QKV
                qt, kt = [], []
                for p in range(8):
                    pq = ps.tile([128, 256], F32, tag="ps", name="ps")
                    pk = ps.tile([128, 256], F32, tag="ps", name="ps")
                    for cb in range(8):
                        st, sp = (cb == 0), (cb == 7)
                        nc.tensor.matmul(pq[:], wq_sb[cb][:, p * 128:(p + 1) * 128],
                                         ht[cb][:], start=st, stop=sp)
                        nc.tensor.matmul(pk[:], wk_sb[cb][:, p * 128:(p + 1) * 128],
                                         ht[cb][:], start=st, stop=sp)
                    tq = qt_p.tile([128, 256], BF16, tag="qt", name="qt")
                    tk = kt_p.tile([128, 256], BF16, tag="kt", name="kt")
                    if p % 2 == 0:
                        nc.vector.tensor_copy(tq[:], pq[:])
                        nc.scalar.copy(tk[:], pk[:])
                    else:
                        nc.scalar.copy(tq[:], pq[:])
                        nc.vector.tensor_copy(tk[:], pk[:])
                    qt.append(tq)
                    kt.append(tk)
                vsb = []
                for sb in range(2):
                    pv = [ps.tile([128, 512], F32, tag="ps", name="ps") for _ in range(2)]
                    for cb in range(8):
                        st, sp = (cb == 0), (cb == 7)
                        for n in range(2):
                            nc.tensor.matmul(
                                pv[n][:], ht[cb][:, sb * 128:(sb + 1) * 128],
                                wv_sb[cb][:, n * 512:(n + 1) * 512],
                                start=st, stop=sp)
                    tv = v_p.tile([128, C], BF16, tag="v", name="v")
                    nc.vector.tensor_copy(tv[:, 0:512], pv[0][:])
                    nc.scalar.copy(tv[:, 512:1024], pv[1][:])
                    vsb.append(tv)
                # attention per head; outputs cat_t[tb] in [t, c] layout
                cat_t = [cat_p.tile([128, C], BF16, tag="cat", name="cat")
                         for _ in range(2)]
                for hh in range(H):
                    pr, off = hh // 2, (hh % 2) * 64
                    qs = qt[pr][off:off + 64, :]
                    ks = kt[pr][off:off + 64, :]
                    hs = slice(hh * 64, (hh + 1) * 64)
                    # scoresT tiles: sc0 = [s0, all t]; sc1 = [s1, t1] only
                    sc0 = ps.tile([128, 256], F32, tag="ps", name="ps")
                    nc.tensor.matmul(sc0[:], ks[:, 0:128], qs[:])
                    # packed bank: sc1 0:128 | po0 128:192 | po1 192:256
                    #              den0 256:257 | den1 257:258
                    pk_t = ps.tile([128, 258], F32, tag="ps", name="ps")
                    nc.tensor.matmul(pk_t[:, 0:128], ks[:, 128:256],
                                     qs[:, 128:256])
                    e0 = ex_p.tile([128, 256], BF16, tag="e0", name="e0")
                    e1 = ex_p.tile([128, 128], BF16, tag="e1", name="e1")
                    nc.scalar.activation(e0[:], sc0[:], AF.Exp, scale=SCALE)
                    nc.scalar.activation(e1[:], pk_t[:, 0:128], AF.Exp,
                                         scale=SCALE)
                    nc.gpsimd.tensor_tensor(e0[:, 0:128], e0[:, 0:128], m0[:],
                                            ALU.mult)
                    nc.gpsimd.tensor_tensor(e1[:], e1[:], m0[:], ALU.mult)
                    # attn@V in [t, d] + denominators via ones column
                    nc.tensor.matmul(pk_t[:, 256:257], e0[:, 0:128], ones[:])
                    nc.tensor.matmul(pk_t[:, 128:192], e0[:, 0:128],
                                     vsb[0][:, hs])
                    nc.tensor.matmul(pk_t[:, 257:258], e0[:, 128:256], ones[:],
                                     start=True, stop=False)
                    nc.tensor.matmul(pk_t[:, 192:256], e0[:, 128:256],
                                     vsb[0][:, hs], start=True, stop=False)
                    nc.tensor.matmul(pk_t[:, 257:258], e1[:], ones[:],
                                     start=False, stop=True)
                    nc.tensor.matmul(pk_t[:, 192:256], e1[:], vsb[1][:, hs],
                                     start=False, stop=True)
                    rec = rec_p.tile([128, 2], F32, tag="rec", name="rec")
                    nc.vector.reciprocal(rec[:, 0:1], pk_t[:, 256:257])
                    nc.vector.reciprocal(rec[:, 1:2], pk_t[:, 257:258])
                    nc.scalar.mul(cat_t[0][:, hs], pk_t[:, 128:192], rec[:, 0:1])
                    nc.scalar.mul(cat_t[1][:, hs], pk_t[:, 192:256], rec[:, 1:2])
                # transpose cat_t -> catT [c, t]
                catT = []
                for cb in range(8):
                    pt = ps.tile([128, 256], BF16, tag="ps", name="ps")
                    for tb in range(2):
                        nc.tensor.transpose(
                            pt[:, tb * 128:(tb + 1) * 128],
                            cat_t[tb][:, cb * 128:(cb + 1) * 128], idb[:])
                    t = ctt_p.tile([128, 256], BF16, tag="ctt", name="ctt")
                    if cb % 2 == 0:
                        nc.vector.tensor_copy(t[:], pt[:])
                    else:
                        nc.scalar.copy(t[:], pt[:])
                    catT.append(t)
                # proj + residual -> x2 -> DRAM spill
                for tb in range(2):
                    x2t = x2_p.tile([128, C], F32, tag="x2", name="x2")
                    for n in range(2):
                        pp = ps.tile([128, 512], F32, tag="ps", name="ps")
                        for cb in range(8):
                            nc.tensor.matmul(
                                pp[:], catT[cb][:, tb * 128:(tb + 1) * 128],
                                wp_sb[cb][:, n * 512:(n + 1) * 512],
                                start=(cb == 0), stop=(cb == 7))
                        nsl = slice(n * 512, (n + 1) * 512)
                        nc.vector.tensor_tensor(x2t[:, nsl], pp[:], xb[tb][:, nsl],
                                                ALU.add)
                        if use_bproj:
                            nc.vector.tensor_tensor(x2t[:, nsl], x2t[:, nsl],
                                                    cb_t["bprojb"][:, nsl], ALU.add)
                    row = b * T + tb * 128
                    nc.sync.dma_start(out=x2_d[row:row + 128, :], in_=x2t[:])

        # ---------------- phase B: FFN ----------------
        with ExitStack() as bctx:
            w1_sb, w2_sb = [], []
            wpoolB = bctx.enter_context(tc.tile_pool(name="wffn", bufs=1))
            for cb in range(8):
                t = wpoolB.tile([128, F4], BF16, tag=f"w1_{cb}", name=f"w1_{cb}")
                nc.sync.dma_start(out=t[:], in_=w1_d[cb * 128:(cb + 1) * 128, :])
                w1_sb.append(t)
            for fb in range(32):
                t = wpoolB.tile([128, C], BF16, tag=f"w2_{fb}", name=f"w2_{fb}")
                nc.sync.dma_start(out=t[:], in_=w2_d[fb * 128:(fb + 1) * 128, :])
                w2_sb.append(t)

            x2B_p = bctx.enter_context(tc.tile_pool(name="x2B", bufs=3))
            ybf_p = bctx.enter_context(tc.tile_pool(name="ybf", bufs=3))
            yt_p = bctx.enter_context(tc.tile_pool(name="yt", bufs=12))
            z1_p = bctx.enter_context(tc.tile_pool(name="z1", bufs=4))
            u_p = bctx.enter_context(tc.tile_pool(name="u", bufs=2))
            psB = bctx.enter_context(tc.tile_pool(name="psB", bufs=2, space="PSUM"))

            for stx in range(NB):
                x2t = [x2B_p.tile([128, C], F32, tag="x2B", name="x2B")
                       for _ in range(2)]
                ybf = [ybf_p.tile([128, C], BF16, tag="ybf", name="ybf")
                       for _ in range(2)]
                for tb in range(2):
                    row = stx * 256 + tb * 128
                    nc.sync.dma_start(out=x2t[tb][:], in_=x2_d[row:row + 128, :])
                    _ln_tile(nc, pools, x2t[tb], ybf[tb],
                             (cb_t["g2b"], cb_t["be2b"]) if aff2 else None)
                ytT = []
                for cb in range(8):
                    pt = psB.tile([128, 256], BF16, tag="pt", name="pt")
                    for tb in range(2):
                        nc.tensor.transpose(
                            pt[:, tb * 128:(tb + 1) * 128],
                            ybf[tb][:, cb * 128:(cb + 1) * 128], idb[:])
                    t = yt_p.tile([128, 256], BF16, tag="yt", name="yt")
                    if cb % 2 == 0:
                        nc.vector.tensor_copy(t[:], pt[:])
                    else:
                        nc.scalar.copy(t[:], pt[:])
                    ytT.append(t)
                z2ps = [psB.tile([128, 512], F32, tag="acc", name="acc", bufs=4)
                        for _ in range(4)]
                for fb in range(32):
                    pz = psB.tile([128, 256], F32, tag="pz", name="pz")
                    for cb in range(8):
                        nc.tensor.matmul(pz[:],
                                         w1_sb[cb][:, fb * 128:(fb + 1) * 128],
                                         ytT[cb][:], start=(cb == 0), stop=(cb == 7))
                    z1 = z1_p.tile([128, 256], BF16, tag="z1", name="z1")
                    if use_b1:
                        nc.scalar.activation(z1[:], pz[:], AF.Relu,
                                             bias=b1t[:, fb:fb + 1])
                    else:
                        nc.scalar.activation(z1[:], pz[:], AF.Relu)
                    for tb in range(2):
                        for n in range(2):
                            nc.tensor.matmul(
                                z2ps[tb * 2 + n][:],
                                z1[:, tb * 128:(tb + 1) * 128],
                                w2_sb[fb][:, n * 512:(n + 1) * 512],
                                start=(fb == 0), stop=(fb == 31))
                for tb in range(2):
                    u = u_p.tile([128, C], F32, tag="u", name="u")
                    for n in range(2):
                        nsl = slice(n * 512, (n + 1) * 512)
                        nc.vector.tensor_tensor(u[:, nsl], z2ps[tb * 2 + n][:],
                                                ybf[tb][:, nsl], ALU.add)
                    if use_b2:
                        nc.vector.tensor_tensor(u[:], u[:], cb_t["b2b"][:],
                                                ALU.add)
                    _ln_tile(nc, pools, u, u,
                             (cb_t["g3b"], cb_t["be3b"]) if aff3 else None)
                    nc.vector.tensor_tensor(x2t[tb][:], x2t[tb][:], u[:], ALU.add)
                    row = stx * 256 + tb * 128
                    nc.sync.dma_start(out=out_d[row:row + 128, :], in_=x2t[tb][:])
    nc.finalize()
    return nc


def _get_nc(flags):
    key = ("nc", flags)
    if key not in _CACHE:
        _CACHE[key] = _build(flags)
    return _CACHE[key]


def kernel(x, wq, wk, wv, w_proj, b_proj, w1, b1, w2, b2,
           g1, be1, g2, be2, g3, be3):
    bf = ml_dtypes.bfloat16
    x = np.asarray(x, np.float32)

    def nz(v):
        return bool(np.any(np.asarray(v, np.float32) != 0.0))

    def naff(g, be):
        return bool(np.any(np.asarray(g, np.float32) != 1.0)) or nz(be)

    flags = (naff(g1, be1), naff(g2, be2), naff(g3, be3),
             nz(b_proj), nz(b1), nz(b2))
    aff1, aff2, aff3, use_bproj, use_b1, use_b2 = flags
    nc = _get_nc(flags)

    def bc(vec):
        return np.ascontiguousarray(
            np.broadcast_to(np.asarray(vec, np.float32).reshape(1, C),
                            (128, C))).astype(bf)

    wqf = np.ascontiguousarray(
        np.asarray(wq, np.float32).transpose(1, 0, 2).reshape(C, C)).astype(bf)
    wkf = np.ascontiguousarray(
        np.asarray(wk, np.float32).transpose(1, 0, 2).reshape(C, C)).astype(bf)
    wvf = np.ascontiguousarray(
        np.asarray(wv, np.float32).transpose(1, 0, 2).reshape(C, C)).astype(bf)
    wpf = np.asarray(w_proj, np.float32).astype(bf)
    w1f = np.asarray(w1, np.float32).astype(bf)
    w2f = np.asarray(w2, np.float32).astype(bf)
    s = np.arange(128)[:, None]
    t = np.arange(128)[None, :]
    m0 = (s <= t).astype(np.float32).astype(bf)
    common = {
        "wqf": wqf, "wkf": wkf, "wvf": wvf, "wpf": wpf,
        "w1f": w1f, "w2f": w2f,
        "mask0": m0,
        "identb": np.eye(128, dtype=np.float32).astype(bf),
        "onesb": np.ones((128, 1), np.float32).astype(bf),
    }
    if use_b1:
        common["b1t"] = np.ascontiguousarray(
            np.asarray(b1, np.float32).reshape(F4 // 128, 128).T)
    if use_bproj:
        common["bprojb"] = bc(b_proj)
    if use_b2:
        common["b2b"] = bc(b2)
    if aff1:
        common["g1b"] = bc(g1)
        common["be1b"] = bc(be1)
    if aff2:
        common["g2b"] = bc(g2)
        common["be2b"] = bc(be2)
    if aff3:
        common["g3b"] = bc(g3)
        common["be3b"] = bc(be3)
    xs = x.reshape(NCORES, TOK, C)
    in_maps = [dict(common, x=np.ascontiguousarray(xs[i]))
               for i in range(NCORES)]
    import os
    trace = bool(os.environ.get("KERNEL_TRACE"))
    res = run_bass_kernel_spmd(nc, in_maps, core_ids=list(range(NCORES)),
                               trace=trace)
    _CACHE["last_res"] = res
    out = np.stack([res.results[i]["out"] for i in range(NCORES)], axis=0)
    return out.reshape(B, T, C).astype(np.float32)


# revision 8
# speedup vs baseline: 1.3566x; 1.0095x over previous
"""Trainium2 Bass kernel for a dense transformer block (B=64,T=256,C=1024,H=16).

Sharding: pure data-parallel over batch across 8 NeuronCores (8 sequences
per core, no collectives). Per-core program:
  phase A (per batch): LN1 (bn_stats) -> h(bf16) -> PE-transpose -> hT ->
  QKV (bf16 matmuls, fp32 accum) -> causal attention with scoresT[s,t]
  tiles; softmax denominator via N=1 matmuls vs ones, attn@V emits
  out[t,d] so the normalization is a per-partition scalar multiply on
  the Scalar engine -> PE-transpose cat back to [c,t] -> proj ->
  residual -> spill x2 to DRAM.
  phase B (per 256-token supertile): LN2 -> yT(bf16) -> FFN (z1T =
  relu(w1.T y) interleaved with z2 accumulation) -> LN3 -> final
  residual -> out.
"""

from contextlib import ExitStack

import ml_dtypes
import numpy as np

import concourse.bass as bass
import concourse.bacc as bacc
import concourse.mybir as mybir
import concourse.tile as tile
from concourse.bass_utils import run_bass_kernel_spmd

F32 = mybir.dt.float32
BF16 = mybir.dt.bfloat16
AF = mybir.ActivationFunctionType
ALU = mybir.AluOpType
AX = mybir.AxisListType

B, T, C, H, D = 64, 256, 1024, 16, 64
NCORES = 8
NB = B // NCORES          # 8 sequences per core
TOK = NB * T              # 2048 tokens per core
F4 = 4 * C                # 4096
EPS = 1e-3
SCALE = C ** -0.5         # 1/32

_CACHE = {}


def _ln_tile(nc, pools, xt, out_t, affine, ncols=C):
    """LayerNorm of one [128, ncols] tile along the free axis via bn_stats.
    out_t may equal xt (in-place). affine = (g_t, be_t) or None."""
    stat = pools["stat"]
    nsub = ncols // 512
    st = stat.tile([128, nsub, 6], F32, tag="bst", name="bst")
    mv = stat.tile([128, 2], F32, tag="bmv", name="bmv")
    rs = stat.tile([128, 1], F32, tag="brs", name="brs")
    xv = xt[:].rearrange("p (a b) -> p a b", b=512)
    for i in range(nsub):
        nc.vector.bn_stats(st[:, i, :], xv[:, i, :])
    nc.vector.bn_aggr(mv[:], st[:])
    nc.scalar.activation(rs[:], mv[:, 1:2], AF.Sqrt, bias=pools["eps"][:])
    nc.vector.reciprocal(rs[:], rs[:])
    nc.vector.tensor_scalar(out_t[:], xt[:], mv[:, 0:1], rs[:],
                            ALU.subtract, ALU.mult)
    if affine is not None:
        g_t, be_t = affine
        nc.vector.tensor_tensor(out_t[:], out_t[:], g_t[:], ALU.mult)
        nc.vector.tensor_tensor(out_t[:], out_t[:], be_t[:], ALU.add)


def _build(flags):
    aff1, aff2, aff3, use_bproj, use_b1, use_b2 = flags
    nc = bacc.Bacc(target_bir_lowering=False)
    x_d = nc.dram_tensor("x", [TOK, C], F32, kind="ExternalInput")
    wq_d = nc.dram_tensor("wqf", [C, C], BF16, kind="ExternalInput")
    wk_d = nc.dram_tensor("wkf", [C, C], BF16, kind="ExternalInput")
    wv_d = nc.dram_tensor("wvf", [C, C], BF16, kind="ExternalInput")
    wp_d = nc.dram_tensor("wpf", [C, C], BF16, kind="ExternalInput")
    w1_d = nc.dram_tensor("w1f", [C, F4], BF16, kind="ExternalInput")
    w2_d = nc.dram_tensor("w2f", [F4, C], BF16, kind="ExternalInput")
    consts_bf = {}
    names = []
    if use_b1:
        b1_d = nc.dram_tensor("b1t", [128, F4 // 128], F32, kind="ExternalInput")
    if use_bproj:
        names.append("bprojb")
    if use_b2:
        names.append("b2b")
    if aff1:
        names += ["g1b", "be1b"]
    if aff2:
        names += ["g2b", "be2b"]
    if aff3:
        names += ["g3b", "be3b"]
    for nm in names:
        consts_bf[nm] = nc.dram_tensor(nm, [128, C], BF16, kind="ExternalInput")
    m0_d = nc.dram_tensor("mask0", [128, 128], BF16, kind="ExternalInput")
    id_d = nc.dram_tensor("identb", [128, 128], BF16, kind="ExternalInput")
    ones_d = nc.dram_tensor("onesb", [128, 1], BF16, kind="ExternalInput")
    out_d = nc.dram_tensor("out", [TOK, C], F32, kind="ExternalOutput")
    x2_d = nc.dram_tensor("x2d", [TOK, C], F32)

    with tile.TileContext(nc) as tc, ExitStack() as ctx:
        const = ctx.enter_context(tc.tile_pool(name="const", bufs=1))
        cb_t = {nm: const.tile([128, C], BF16, tag=nm, name=nm)
                for nm in consts_bf}
        for nm, t in cb_t.items():
            nc.sync.dma_start(out=t[:], in_=consts_bf[nm][:, :])
        m0 = const.tile([128, 128], BF16, tag="m0", name="m0")
        nc.sync.dma_start(out=m0[:], in_=m0_d[:, :])
        idb = const.tile([128, 128], BF16, tag="idb", name="idb")
        nc.sync.dma_start(out=idb[:], in_=id_d[:, :])
        ones = const.tile([128, 1], BF16, tag="ones", name="ones")
        nc.sync.dma_start(out=ones[:], in_=ones_d[:, :])
        if use_b1:
            b1t = const.tile([128, F4 // 128], F32, tag="b1t", name="b1t")
            nc.sync.dma_start(out=b1t[:], in_=b1_d[:, :])

        epsb = const.tile([128, 1], F32, tag="eps", name="eps")
        nc.gpsimd.memset(epsb[:], EPS)

        stat = ctx.enter_context(tc.tile_pool(name="stat", bufs=8))
        pools = {"stat": stat, "eps": epsb}

        # ---------------- phase A: attention ----------------
        with ExitStack() as actx:
            wpool = actx.enter_context(tc.tile_pool(name="wqkv", bufs=1))
            wq_sb, wk_sb, wv_sb, wp_sb = [], [], [], []
            for cb in range(8):
                for lst, dram, nm in ((wq_sb, wq_d, "wq"), (wk_sb, wk_d, "wk"),
                                      (wv_sb, wv_d, "wv"), (wp_sb, wp_d, "wp")):
                    t = wpool.tile([128, C], BF16, tag=f"{nm}{cb}", name=f"{nm}{cb}")
                    nc.sync.dma_start(out=t[:], in_=dram[cb * 128:(cb + 1) * 128, :])
                    lst.append(t)

            xb_p = actx.enter_context(tc.tile_pool(name="xb", bufs=4))
            h_p = actx.enter_context(tc.tile_pool(name="h", bufs=4))
            ht_p = actx.enter_context(tc.tile_pool(name="ht", bufs=12))
            qt_p = actx.enter_context(tc.tile_pool(name="qt", bufs=12))
            kt_p = actx.enter_context(tc.tile_pool(name="kt", bufs=12))
            v_p = actx.enter_context(tc.tile_pool(name="v", bufs=4))
            ex_p = actx.enter_context(tc.tile_pool(name="ex", bufs=8))
            cat_p = actx.enter_context(tc.tile_pool(name="cat", bufs=4))
            ctt_p = actx.enter_context(tc.tile_pool(name="ctt", bufs=10))
            rec_p = actx.enter_context(tc.tile_pool(name="rec", bufs=12))
            x2_p = actx.enter_context(tc.tile_pool(name="x2", bufs=4))
            ps = actx.enter_context(tc.tile_pool(name="psA", bufs=8, space="PSUM"))

            for b in range(NB):
                xb = [xb_p.tile([128, C], F32, tag="xb", name="xb") for _ in range(2)]
                hbf = [h_p.tile([128, C], BF16, tag="h", name="h") for _ in range(2)]
                for tb in range(2):
                    row = b * T + tb * 128
                    nc.sync.dma_start(out=xb[tb][:], in_=x_d[row:row + 128, :])
                    _ln_tile(nc, pools, xb[tb], hbf[tb],
                             (cb_t["g1b"], cb_t["be1b"]) if aff1 else None)
                # transpose h -> hT (8 tiles [128c, 256t] bf16)
                ht = []
                for cb in range(8):
                    pt = ps.tile([128, 256], BF16, tag="ps", name="ps")
                    for tb in range(2):
                        nc.tensor.transpose(
                            pt[:, tb * 128:(tb + 1) * 128],
                            hbf[tb][:, cb * 128:(cb + 1) * 128], idb[:])
                    t = ht_p.tile([128, 256], BF16, tag="ht", name="ht")
                    if cb % 2 == 0:
                        nc.vector.tensor_copy(t[:], pt[:])
                    else:
                        nc.scalar.copy(t[:], pt[:])
                    ht.append(t)
                # QKV
                qt, kt = [], []
                for p in range(8):
                    pq = ps.tile([128, 256], F32, tag="ps", name="ps")
                    pk = ps.tile([128, 256], F32, tag="ps", name="ps")
                    for cb in range(8):
                        st, sp = (cb == 0), (cb == 7)
                        nc.tensor.matmul(pq[:], wq_sb[cb][:, p * 128:(p + 1) * 128],
                                         ht[cb][:], start=st, stop=sp)
                        nc.tensor.matmul(pk[:], wk_sb[cb][:, p * 128:(p + 1) * 128],
                                         ht[cb][:], start=st, stop=sp)
                    tq = qt_p.tile([128, 256], BF16, tag="qt", name="qt")
                    tk = kt_p.tile([128, 256], BF16, tag="kt", name="kt")
                    if p % 2 == 0:
                        nc.vector.tensor_copy(tq[:], pq[:])
                        nc.scalar.copy(tk[:], pk[:])
                    else:
                        nc.scalar.copy(tq[:], pq[:])
                        nc.vector.tensor_copy(tk[:], pk[:])
                    qt.append(tq)
                    kt.append(tk)
                vsb = []
                for sb in range(2):
                    pv = [ps.tile([128, 512], F32, tag="ps", name="ps") for _ in range(2)]
                    for cb in range(8):
                        st, sp = (cb == 0), (cb == 7)
                        for n in range(2):
                            nc.tensor.matmul(
                                pv[n][:], ht[cb][:, sb * 128:(sb + 1) * 128],
                                wv_sb[cb][:, n * 512:(n + 1) * 512],
                                start=st, stop=sp)
                    tv = v_p.tile([128, C], BF16, tag="v", name="v")
                    nc.vector.tensor_copy(tv[:, 0:512], pv[0][:])
                    nc.scalar.copy(tv[:, 512:1024], pv[1][:])
                    vsb.append(tv)
                # attention per head; outputs cat_t[tb] in [t, c] layout
                cat_t = [cat_p.tile([128, C], BF16, tag="cat", name="cat")
                         for _ in range(2)]
                for hh in range(H):
                    pr, off = hh // 2, (hh % 2) * 64
                    qs = qt[pr][off:off + 64, :]
                    ks = kt[pr][off:off + 64, :]
                    hs = slice(hh * 64, (hh + 1) * 64)
                    # scoresT tiles: sc0 = [s0, all t]; sc1 = [s1, t1] only
                    sc0 = ps.tile([128, 256], F32, tag="ps", name="ps")
                    nc.tensor.matmul(sc0[:], ks[:, 0:128], qs[:])
                    sc1 = ps.tile([128, 128], F32, tag="ps", name="ps")
                    nc.tensor.matmul(sc1[:], ks[:, 128:256], qs[:, 128:256])
                    e0 = ex_p.tile([128, 256], BF16, tag="e0", name="e0")
                    e1 = ex_p.tile([128, 128], BF16, tag="e1", name="e1")
                    nc.scalar.activation(e0[:], sc0[:], AF.Exp, scale=SCALE)
                    nc.scalar.activation(e1[:], sc1[:], AF.Exp, scale=SCALE)
                    nc.gpsimd.tensor_tensor(e0[:, 0:128], e0[:, 0:128], m0[:],
                                            ALU.mult)
                    nc.gpsimd.tensor_tensor(e1[:], e1[:], m0[:], ALU.mult)
                    # attn@V in [t, d] + denominators, packed in one bank:
                    # po0 0:64 | po1 64:128 | den0 128:129 | den1 129:130.
                    # One accumulation chain: start clears has_written for the
                    # whole bank, so only the first matmul starts; later ones
                    # fresh-write their region (bit clear) or accumulate.
                    att = ps.tile([128, 130], F32, tag="ps", name="ps")
                    nc.tensor.matmul(att[:, 0:64], e0[:, 0:128], vsb[0][:, hs],
                                     start=True, stop=False,
                                     skip_group_check=True)
                    nc.tensor.matmul(att[:, 128:129], e0[:, 0:128], ones[:],
                                     start=False, stop=False,
                                     skip_group_check=True)
                    nc.tensor.matmul(att[:, 64:128], e0[:, 128:256],
                                     vsb[0][:, hs], start=False, stop=False,
                                     skip_group_check=True)
                    nc.tensor.matmul(att[:, 129:130], e0[:, 128:256], ones[:],
                                     start=False, stop=False,
                                     skip_group_check=True)
                    nc.tensor.matmul(att[:, 64:128], e1[:], vsb[1][:, hs],
                                     start=False, stop=False,
                                     skip_group_check=True)
                    nc.tensor.matmul(att[:, 129:130], e1[:], ones[:],
                                     start=False, stop=True,
                                     skip_group_check=True)
                    # one reciprocal over both denominators: RAW on the last
                    # matmul of the bank, so every later read of this bank
                    # (the scalar muls wait on rec) follows all PE writes.
                    rec = rec_p.tile([128, 2], F32, tag="rec", name="rec")
                    nc.vector.reciprocal(rec[:], att[:, 128:130])
                    nc.scalar.mul(cat_t[0][:, hs], att[:, 0:64], rec[:, 0:1])
                    nc.scalar.mul(cat_t[1][:, hs], att[:, 64:128], rec[:, 1:2])
                # transpose cat_t -> catT [c, t]
                catT = []
                for cb in range(8):
                    pt = ps.tile([128, 256], BF16, tag="ps", name="ps")
                    for tb in range(2):
                        nc.tensor.transpose(
                            pt[:, tb * 128:(tb + 1) * 128],
                            cat_t[tb][:, cb * 128:(cb + 1) * 128], idb[:])
                    t = ctt_p.tile([128, 256], BF16, tag="ctt", name="ctt")
                    if cb % 2 == 0:
                        nc.vector.tensor_copy(t[:], pt[:])
                    else:
                        nc.scalar.copy(t[:], pt[:])
                    catT.append(t)
                # proj + residual -> x2 -> DRAM spill
                for tb in range(2):
                    x2t = x2_p.tile([128, C], F32, tag="x2", name="x2")
                    for n in range(2):
                        pp = ps.tile([128, 512], F32, tag="ps", name="ps")
                        for cb in range(8):
                            nc.tensor.matmul(
                                pp[:], catT[cb][:, tb * 128:(tb + 1) * 128],
                                wp_sb[cb][:, n * 512:(n + 1) * 512],
                                start=(cb == 0), stop=(cb == 7))
                        nsl = slice(n * 512, (n + 1) * 512)
                        nc.vector.tensor_tensor(x2t[:, nsl], pp[:], xb[tb][:, nsl],
                                                ALU.add)
                        if use_bproj:
                            nc.vector.tensor_tensor(x2t[:, nsl], x2t[:, nsl],
                                                    cb_t["bprojb"][:, nsl], ALU.add)
                    row = b * T + tb * 128
                    nc.sync.dma_start(out=x2_d[row:row + 128, :], in_=x2t[:])

        # ---------------- phase B: FFN ----------------
        with ExitStack() as bctx:
            w1_sb, w2_sb = [], []
            wpoolB = bctx.enter_context(tc.tile_pool(name="wffn", bufs=1))
            for cb in range(8):
                t = wpoolB.tile([128, F4], BF16, tag=f"w1_{cb}", name=f"w1_{cb}")
                nc.sync.dma_start(out=t[:], in_=w1_d[cb * 128:(cb + 1) * 128, :])
                w1_sb.append(t)
            for fb in range(32):
                t = wpoolB.tile([128, C], BF16, tag=f"w2_{fb}", name=f"w2_{fb}")
                nc.sync.dma_start(out=t[:], in_=w2_d[fb * 128:(fb + 1) * 128, :])
                w2_sb.append(t)

            x2B_p = bctx.enter_context(tc.tile_pool(name="x2B", bufs=3))
            ybf_p = bctx.enter_context(tc.tile_pool(name="ybf", bufs=3))
            yt_p = bctx.enter_context(tc.tile_pool(name="yt", bufs=12))
            z1_p = bctx.enter_context(tc.tile_pool(name="z1", bufs=4))
            u_p = bctx.enter_context(tc.tile_pool(name="u", bufs=2))
            psB = bctx.enter_context(tc.tile_pool(name="psB", bufs=2, space="PSUM"))

            for stx in range(NB):
                x2t = [x2B_p.tile([128, C], F32, tag="x2B", name="x2B")
                       for _ in range(2)]
                ybf = [ybf_p.tile([128, C], BF16, tag="ybf", name="ybf")
                       for _ in range(2)]
                for tb in range(2):
                    row = stx * 256 + tb * 128
                    nc.sync.dma_start(out=x2t[tb][:], in_=x2_d[row:row + 128, :])
                    _ln_tile(nc, pools, x2t[tb], ybf[tb],
                             (cb_t["g2b"], cb_t["be2b"]) if aff2 else None)
                ytT = []
                for cb in range(8):
                    pt = psB.tile([128, 256], BF16, tag="pt", name="pt")
                    for tb in range(2):
                        nc.tensor.transpose(
                            pt[:, tb * 128:(tb + 1) * 128],
                            ybf[tb][:, cb * 128:(cb + 1) * 128], idb[:])
                    t = yt_p.tile([128, 256], BF16, tag="yt", name="yt")
                    if cb % 2 == 0:
                        nc.vector.tensor_copy(t[:], pt[:])
                    else:
                        nc.scalar.copy(t[:], pt[:])
                    ytT.append(t)
                z2ps = [psB.tile([128, 512], F32, tag="acc", name="acc", bufs=4)
                        for _ in range(4)]
                for fb in range(32):
                    pz = psB.tile([128, 256], F32, tag="pz", name="pz")
                    for cb in range(8):
                        nc.tensor.matmul(pz[:],
                                         w1_sb[cb][:, fb * 128:(fb + 1) * 128],
                                         ytT[cb][:], start=(cb == 0), stop=(cb == 7))
                    z1 = z1_p.tile([128, 256], BF16, tag="z1", name="z1")
                    if use_b1:
                        nc.scalar.activation(z1[:], pz[:], AF.Relu,
                                             bias=b1t[:, fb:fb + 1])
                    else:
                        nc.scalar.activation(z1[:], pz[:], AF.Relu)
                    for tb in range(2):
                        for n in range(2):
                            nc.tensor.matmul(
                                z2ps[tb * 2 + n][:],
                                z1[:, tb * 128:(tb + 1) * 128],
                                w2_sb[fb][:, n * 512:(n + 1) * 512],
                                start=(fb == 0), stop=(fb == 31))
                for tb in range(2):
                    u = u_p.tile([128, C], F32, tag="u", name="u")
                    for n in range(2):
                        nsl = slice(n * 512, (n + 1) * 512)
                        nc.vector.tensor_tensor(u[:, nsl], z2ps[tb * 2 + n][:],
                                                ybf[tb][:, nsl], ALU.add)
                    if use_b2:
                        nc.vector.tensor_tensor(u[:], u[:], cb_t["b2b"][:],
                                                ALU.add)
                    _ln_tile(nc, pools, u, u,
                             (cb_t["g3b"], cb_t["be3b"]) if aff3 else None)
                    nc.vector.tensor_tensor(x2t[tb][:], x2t[tb][:], u[:], ALU.add)
                    row = stx * 256 + tb * 128
                    nc.sync.dma_start(out=out_d[row:row + 128, :], in_=x2t[tb][:])
    nc.finalize()
    return nc


def _get_nc(flags):
    key = ("nc", flags)
    if key not in _CACHE:
        _CACHE[key] = _build(flags)
    return _CACHE[key]


def kernel(x, wq, wk, wv, w_proj, b_proj, w1, b1, w2, b2,
           g1, be1, g2, be2, g3, be3):
    bf = ml_dtypes.bfloat16
    x = np.asarray(x, np.float32)

    def nz(v):
        return bool(np.any(np.asarray(v, np.float32) != 0.0))

    def naff(g, be):
        return bool(np.any(np.asarray(g, np.float32) != 1.0)) or nz(be)

    flags = (naff(g1, be1), naff(g2, be2), naff(g3, be3),
             nz(b_proj), nz(b1), nz(b2))
    aff1, aff2, aff3, use_bproj, use_b1, use_b2 = flags
    nc = _get_nc(flags)

    def bc(vec):
        return np.ascontiguousarray(
            np.broadcast_to(np.asarray(vec, np.float32).reshape(1, C),
                            (128, C))).astype(bf)

    wqf = np.ascontiguousarray(
        np.asarray(wq, np.float32).transpose(1, 0, 2).reshape(C, C)).astype(bf)
    wkf = np.ascontiguousarray(
        np.asarray(wk, np.float32).transpose(1, 0, 2).reshape(C, C)).astype(bf)
    wvf = np.ascontiguousarray(
        np.asarray(wv, np.float32).transpose(1, 0, 2).reshape(C, C)).astype(bf)
    wpf = np.asarray(w_proj, np.float32).astype(bf)
    w1f = np.asarray(w1, np.float32).astype(bf)
    w2f = np.asarray(w2, np.float32).astype(bf)
    s = np.arange(128)[:, None]
    t = np.arange(128)[None, :]
    m0 = (s <= t).astype(np.float32).astype(bf)
    common = {
        "wqf": wqf, "wkf": wkf, "wvf": wvf, "wpf": wpf,
        "w1f": w1f, "w2f": w2f,
        "mask0": m0,
        "identb": np.eye(128, dtype=np.float32).astype(bf),
        "onesb": np.ones((128, 1), np.float32).astype(bf),
    }
    if use_b1:
        common["b1t"] = np.ascontiguousarray(
            np.asarray(b1, np.float32).reshape(F4 // 128, 128).T)
    if use_bproj:
        common["bprojb"] = bc(b_proj)
    if use_b2:
        common["b2b"] = bc(b2)
    if aff1:
        common["g1b"] = bc(g1)
        common["be1b"] = bc(be1)
    if aff2:
        common["g2b"] = bc(g2)
        common["be2b"] = bc(be2)
    if aff3:
        common["g3b"] = bc(g3)
        common["be3b"] = bc(be3)
    xs = x.reshape(NCORES, TOK, C)
    in_maps = [dict(common, x=np.ascontiguousarray(xs[i]))
               for i in range(NCORES)]
    import os
    trace = bool(os.environ.get("KERNEL_TRACE"))
    res = run_bass_kernel_spmd(nc, in_maps, core_ids=list(range(NCORES)),
                               trace=trace)
    _CACHE["last_res"] = res
    out = np.stack([res.results[i]["out"] for i in range(NCORES)], axis=0)
    return out.reshape(B, T, C).astype(np.float32)


# revision 17
# speedup vs baseline: 1.4516x; 1.0701x over previous
"""Trainium2 Bass kernel for a dense transformer block (B=64,T=256,C=1024,H=16).

Sharding: pure data-parallel over batch across 8 NeuronCores (8 sequences
per core, no collectives). Per-core program:
  phase A (per batch): LN1 (bn_stats) -> h(bf16) -> PE-transpose -> hT ->
  QKV (bf16 matmuls, fp32 accum) -> causal attention with scoresT[s,t]
  tiles; softmax denominator via N=1 matmuls vs ones, attn@V emits
  out[t,d] so the normalization is a per-partition scalar multiply on
  the Scalar engine -> PE-transpose cat back to [c,t] -> proj ->
  residual -> spill x2 to DRAM.
  phase B (per 256-token supertile): LN2 -> yT(bf16) -> FFN (z1T =
  relu(w1.T y) interleaved with z2 accumulation) -> LN3 -> final
  residual -> out.
"""

from contextlib import ExitStack

import ml_dtypes
import numpy as np

import concourse.bass as bass
import concourse.bacc as bacc
import concourse.mybir as mybir
import concourse.tile as tile
from concourse.bass_utils import run_bass_kernel_spmd

F32 = mybir.dt.float32
BF16 = mybir.dt.bfloat16
AF = mybir.ActivationFunctionType
ALU = mybir.AluOpType
AX = mybir.AxisListType

B, T, C, H, D = 64, 256, 1024, 16, 64
NCORES = 8
NB = B // NCORES          # 8 sequences per core
TOK = NB * T              # 2048 tokens per core
F4 = 4 * C                # 4096
EPS = 1e-3
SCALE = C ** -0.5         # 1/32

_CACHE = {}


def _ln_tile(nc, pools, xt, out_t, affine, ncols=C):
    """LayerNorm of one [128, ncols] tile along the free axis via bn_stats.
    out_t may equal xt (in-place). affine = (g_t, be_t) or None."""
    stat = pools["stat"]
    nsub = ncols // 512
    st = stat.tile([128, nsub, 6], F32, tag="bst", name="bst")
    mv = stat.tile([128, 2], F32, tag="bmv", name="bmv")
    rs = stat.tile([128, 1], F32, tag="brs", name="brs")
    xv = xt[:].rearrange("p (a b) -> p a b", b=512)
    for i in range(nsub):
        nc.vector.bn_stats(st[:, i, :], xv[:, i, :])
    nc.vector.bn_aggr(mv[:], st[:])
    nc.scalar.activation(rs[:], mv[:, 1:2], AF.Sqrt, bias=pools["eps"][:])
    nc.vector.reciprocal(rs[:], rs[:])
    nc.vector.tensor_scalar(out_t[:], xt[:], mv[:, 0:1], rs[:],
                            ALU.subtract, ALU.mult)
    if affine is not None:
        g_t, be_t = affine
        nc.vector.tensor_tensor(out_t[:], out_t[:], g_t[:], ALU.mult)
        nc.vector.tensor_tensor(out_t[:], out_t[:], be_t[:], ALU.add)


def _build(flags):
    aff1, aff2, aff3, use_bproj, use_b1, use_b2 = flags
    nc = bacc.Bacc(target_bir_lowering=False)
    x_d = nc.dram_tensor("x", [TOK, C], F32, kind="ExternalInput")
    wq_d = nc.dram_tensor("wqf", [C, C], BF16, kind="ExternalInput")
    wk_d = nc.dram_tensor("wkf", [C, C], BF16, kind="ExternalInput")
    wv_d = nc.dram_tensor("wvf", [C, C], BF16, kind="ExternalInput")
    wp_d = nc.dram_tensor("wpf", [C, C], BF16, kind="ExternalInput")
    w1_d = nc.dram_tensor("w1f", [C, F4], BF16, kind="ExternalInput")
    w2_d = nc.dram_tensor("w2f", [F4, C], BF16, kind="ExternalInput")
    consts_bf = {}
    names = []
    if use_b1:
        b1_d = nc.dram_tensor("b1t", [128, F4 // 128], F32, kind="ExternalInput")
    if use_bproj:
        names.append("bprojb")
    if use_b2:
        names.append("b2b")
    if aff1:
        names += ["g1b", "be1b"]
    if aff2:
        names += ["g2b", "be2b"]
    if aff3:
        names += ["g3b", "be3b"]
    for nm in names:
        consts_bf[nm] = nc.dram_tensor(nm, [128, C], BF16, kind="ExternalInput")
    m0_d = nc.dram_tensor("mask0", [128, 128], BF16, kind="ExternalInput")
    id_d = nc.dram_tensor("identb", [128, 128], BF16, kind="ExternalInput")
    ones_d = nc.dram_tensor("onesb", [128, 1], BF16, kind="ExternalInput")
    out_d = nc.dram_tensor("out", [TOK, C], F32, kind="ExternalOutput")
    x2_d = nc.dram_tensor("x2d", [TOK, C], F32)

    with tile.TileContext(nc) as tc, ExitStack() as ctx:
        const = ctx.enter_context(tc.tile_pool(name="const", bufs=1))
        cb_t = {nm: const.tile([128, C], BF16, tag=nm, name=nm)
                for nm in consts_bf}
        for nm, t in cb_t.items():
            nc.sync.dma_start(out=t[:], in_=consts_bf[nm][:, :])
        m0 = const.tile([128, 128], BF16, tag="m0", name="m0")
        nc.sync.dma_start(out=m0[:], in_=m0_d[:, :])
        idb = const.tile([128, 128], BF16, tag="idb", name="idb")
        nc.sync.dma_start(out=idb[:], in_=id_d[:, :])
        ones = const.tile([128, 1], BF16, tag="ones", name="ones")
        nc.sync.dma_start(out=ones[:], in_=ones_d[:, :])
        if use_b1:
            b1t = const.tile([128, F4 // 128], F32, tag="b1t", name="b1t")
            nc.sync.dma_start(out=b1t[:], in_=b1_d[:, :])

        epsb = const.tile([128, 1], F32, tag="eps", name="eps")
        nc.gpsimd.memset(epsb[:], EPS)

        stat = ctx.enter_context(tc.tile_pool(name="stat", bufs=8))
        pools = {"stat": stat, "eps": epsb}

        # ---------------- phase A: attention ----------------
        with ExitStack() as actx:
            xb_p = actx.enter_context(tc.tile_pool(name="xb", bufs=4))
            xb_tiles = {}

            def load_x(b):
                if b >= NB:
                    return
                ts = [xb_p.tile([128, C], F32, tag="xb", name="xb")
                      for _ in range(2)]
                for tb in range(2):
                    row = b * T + tb * 128
                    nc.sync.dma_start(out=ts[tb][:], in_=x_d[row:row + 128, :])
                xb_tiles[b] = ts

            # batch-0 input loads go on the queue before the weight DMAs so
            # LN1 can start while weights stream in.
            load_x(0)

            wpool = actx.enter_context(tc.tile_pool(name="wqkv", bufs=1))
            wq_sb, wk_sb, wv_sb, wp_sb = [], [], [], []
            for cb in range(8):
                for lst, dram, nm in ((wq_sb, wq_d, "wq"), (wk_sb, wk_d, "wk"),
                                      (wv_sb, wv_d, "wv"), (wp_sb, wp_d, "wp")):
                    t = wpool.tile([128, C], BF16, tag=f"{nm}{cb}", name=f"{nm}{cb}")
                    nc.sync.dma_start(out=t[:], in_=dram[cb * 128:(cb + 1) * 128, :])
                    lst.append(t)
            h_p = actx.enter_context(tc.tile_pool(name="h", bufs=4))
            ht_p = actx.enter_context(tc.tile_pool(name="ht", bufs=12))
            qt_p = actx.enter_context(tc.tile_pool(name="qt", bufs=12))
            kt_p = actx.enter_context(tc.tile_pool(name="kt", bufs=12))
            v_p = actx.enter_context(tc.tile_pool(name="v", bufs=4))
            ex_p = actx.enter_context(tc.tile_pool(name="ex", bufs=8))
            cat_p = actx.enter_context(tc.tile_pool(name="cat", bufs=4))
            ctt_p = actx.enter_context(tc.tile_pool(name="ctt", bufs=10))
            rec_p = actx.enter_context(tc.tile_pool(name="rec", bufs=12))
            x2_p = actx.enter_context(tc.tile_pool(name="x2", bufs=4))
            ps = actx.enter_context(tc.tile_pool(name="psA", bufs=8, space="PSUM"))

            for b in range(NB):
                load_x(b + 1)
                xb = xb_tiles.pop(b)
                hbf = [h_p.tile([128, C], BF16, tag="h", name="h") for _ in range(2)]
                for tb in range(2):
                    _ln_tile(nc, pools, xb[tb], hbf[tb],
                             (cb_t["g1b"], cb_t["be1b"]) if aff1 else None)
                # transpose h -> hT (8 tiles [128c, 256t] bf16)
                ht = []
                for cb in range(8):
                    pt = ps.tile([128, 256], BF16, tag="ps", name="ps")
                    for tb in range(2):
                        nc.tensor.transpose(
                            pt[:, tb * 128:(tb + 1) * 128],
                            hbf[tb][:, cb * 128:(cb + 1) * 128], idb[:])
                    t = ht_p.tile([128, 256], BF16, tag="ht", name="ht")
                    nc.vector.tensor_copy(t[:], pt[:])
                    ht.append(t)
                # QKV
                qt, kt = [], []
                for p in range(8):
                    pq = ps.tile([128, 256], F32, tag="ps", name="ps")
                    pk = ps.tile([128, 256], F32, tag="ps", name="ps")
                    for cb in range(8):
                        st, sp = (cb == 0), (cb == 7)
                        nc.tensor.matmul(pq[:], wq_sb[cb][:, p * 128:(p + 1) * 128],
                                         ht[cb][:], start=st, stop=sp)
                        nc.tensor.matmul(pk[:], wk_sb[cb][:, p * 128:(p + 1) * 128],
                                         ht[cb][:], start=st, stop=sp)
                    tq = qt_p.tile([128, 256], BF16, tag="qt", name="qt")
                    tk = kt_p.tile([128, 256], BF16, tag="kt", name="kt")
                    nc.vector.tensor_copy(tq[:], pq[:])
                    nc.vector.tensor_copy(tk[:], pk[:])
                    qt.append(tq)
                    kt.append(tk)
                vsb = []
                for sb in range(2):
                    pv = [ps.tile([128, 512], F32, tag="ps", name="ps") for _ in range(2)]
                    for cb in range(8):
                        for q4 in range(4):
                            nc.tensor.matmul(
                                pv[q4 // 2][:, (q4 % 2) * 256:(q4 % 2) * 256 + 256],
                                ht[cb][:, sb * 128:(sb + 1) * 128],
                                wv_sb[cb][:, q4 * 256:(q4 + 1) * 256],
                                start=(cb == 0 and q4 % 2 == 0),
                                stop=(cb == 7 and q4 % 2 == 1),
                                skip_group_check=True)
                    tv = v_p.tile([128, C], BF16, tag="v", name="v")
                    nc.vector.tensor_copy(tv[:, 0:512], pv[0][:])
                    nc.vector.tensor_copy(tv[:, 512:1024], pv[1][:])
                    vsb.append(tv)
                # attention per head; outputs cat_t[tb] in [t, c] layout
                cat_t = [cat_p.tile([128, C], BF16, tag="cat", name="cat")
                         for _ in range(2)]
                for hh in range(H):
                    pr, off = hh // 2, (hh % 2) * 64
                    qs = qt[pr][off:off + 64, :]
                    ks = kt[pr][off:off + 64, :]
                    hs = slice(hh * 64, (hh + 1) * 64)
                    # scoresT tiles: sc0 = [s0, all t]; sc1 = [s1, t1] only
                    sc0 = ps.tile([128, 256], F32, tag="ps", name="ps")
                    nc.tensor.matmul(sc0[:], ks[:, 0:128], qs[:])
                    sc1 = ps.tile([128, 128], F32, tag="ps", name="ps")
                    nc.tensor.matmul(sc1[:], ks[:, 128:256], qs[:, 128:256])
                    e0 = ex_p.tile([128, 256], BF16, tag="e0", name="e0")
                    e1 = ex_p.tile([128, 128], BF16, tag="e1", name="e1")
                    nc.scalar.activation(e0[:], sc0[:], AF.Exp, scale=SCALE)
                    nc.scalar.activation(e1[:], sc1[:], AF.Exp, scale=SCALE)
                    nc.gpsimd.tensor_tensor(e0[:, 0:128], e0[:, 0:128], m0[:],
                                            ALU.mult)
                    nc.gpsimd.tensor_tensor(e1[:], e1[:], m0[:], ALU.mult)
                    # attn@V in [t, d] + denominators, packed in one bank:
                    # po0 0:64 | po1 64:128 | den0 128:129 | den1 129:130.
                    # One accumulation chain: start clears has_written for the
                    # whole bank, so only the first matmul starts; later ones
                    # fresh-write their region (bit clear) or accumulate.
                    att = ps.tile([128, 130], F32, tag="ps", name="ps")
                    nc.tensor.matmul(att[:, 0:64], e0[:, 0:128], vsb[0][:, hs],
                                     start=True, stop=False,
                                     skip_group_check=True)
                    nc.tensor.matmul(att[:, 128:129], e0[:, 0:128], ones[:],
                                     start=False, stop=False,
                                     skip_group_check=True)
                    nc.tensor.matmul(att[:, 64:128], e0[:, 128:256],
                                     vsb[0][:, hs], start=False, stop=False,
                                     skip_group_check=True)
                    nc.tensor.matmul(att[:, 129:130], e0[:, 128:256], ones[:],
                                     start=False, stop=False,
                                     skip_group_check=True)
                    nc.tensor.matmul(att[:, 64:128], e1[:], vsb[1][:, hs],
                                     start=False, stop=False,
                                     skip_group_check=True)
                    nc.tensor.matmul(att[:, 129:130], e1[:], ones[:],
                                     start=False, stop=True,
                                     skip_group_check=True)
                    # one reciprocal over both denominators: RAW on the last
                    # matmul of the bank, so every later read of this bank
                    # (the scalar muls wait on rec) follows all PE writes.
                    rec = rec_p.tile([128, 2], F32, tag="rec", name="rec")
                    nc.vector.reciprocal(rec[:], att[:, 128:130])
                    nc.scalar.mul(cat_t[0][:, hs], att[:, 0:64], rec[:, 0:1])
                    nc.scalar.mul(cat_t[1][:, hs], att[:, 64:128], rec[:, 1:2])
                # transpose cat_t -> catT [c, t]
                catT = []
                for cb in range(8):
                    pt = ps.tile([128, 256], BF16, tag="ps", name="ps")
                    for tb in range(2):
                        nc.tensor.transpose(
                            pt[:, tb * 128:(tb + 1) * 128],
                            cat_t[tb][:, cb * 128:(cb + 1) * 128], idb[:])
                    t = ctt_p.tile([128, 256], BF16, tag="ctt", name="ctt")
                    if cb % 2 == 0:
                        nc.vector.tensor_copy(t[:], pt[:])
                    else:
                        nc.scalar.copy(t[:], pt[:])
                    catT.append(t)
                # proj + residual -> x2 -> DRAM spill
                for tb in range(2):
                    x2t = x2_p.tile([128, C], F32, tag="x2", name="x2")
                    for n in range(2):
                        pp = ps.tile([128, 512], F32, tag="ps", name="ps")
                        for cb in range(8):
                            for nh in range(2):
                                nc.tensor.matmul(
                                    pp[:, nh * 256:(nh + 1) * 256],
                                    catT[cb][:, tb * 128:(tb + 1) * 128],
                                    wp_sb[cb][:, n * 512 + nh * 256:
                                               n * 512 + (nh + 1) * 256],
                                    start=(cb == 0 and nh == 0),
                                    stop=(cb == 7 and nh == 1),
                                    skip_group_check=True)
                        nsl = slice(n * 512, (n + 1) * 512)
                        nc.vector.tensor_tensor(x2t[:, nsl], pp[:], xb[tb][:, nsl],
                                                ALU.add)
                        if use_bproj:
                            nc.vector.tensor_tensor(x2t[:, nsl], x2t[:, nsl],
                                                    cb_t["bprojb"][:, nsl], ALU.add)
                    row = b * T + tb * 128
                    nc.sync.dma_start(out=x2_d[row:row + 128, :], in_=x2t[:])

        # ---------------- phase B: FFN ----------------
        with ExitStack() as bctx:
            x2B_p = bctx.enter_context(tc.tile_pool(name="x2B", bufs=4))
            x2_tiles = {}

            def load_x2(stx):
                if stx >= NB:
                    return
                ts = [x2B_p.tile([128, C], F32, tag="x2B", name="x2B")
                      for _ in range(2)]
                for tb in range(2):
                    row = stx * 256 + tb * 128
                    nc.sync.dma_start(out=ts[tb][:], in_=x2_d[row:row + 128, :])
                x2_tiles[stx] = ts

            # first supertile's loads precede the FFN weight DMAs on the queue
            load_x2(0)

            w1_sb, w2_sb = [], []
            wpoolB = bctx.enter_context(tc.tile_pool(name="wffn", bufs=1))
            for cb in range(8):
                t = wpoolB.tile([128, F4], BF16, tag=f"w1_{cb}", name=f"w1_{cb}")
                nc.sync.dma_start(out=t[:], in_=w1_d[cb * 128:(cb + 1) * 128, :])
                w1_sb.append(t)
            for fb in range(32):
                t = wpoolB.tile([128, C], BF16, tag=f"w2_{fb}", name=f"w2_{fb}")
                nc.sync.dma_start(out=t[:], in_=w2_d[fb * 128:(fb + 1) * 128, :])
                w2_sb.append(t)
            ybf_p = bctx.enter_context(tc.tile_pool(name="ybf", bufs=3))
            yt_p = bctx.enter_context(tc.tile_pool(name="yt", bufs=12))
            z1_p = bctx.enter_context(tc.tile_pool(name="z1", bufs=4))
            u_p = bctx.enter_context(tc.tile_pool(name="u", bufs=2))
            psB = bctx.enter_context(tc.tile_pool(name="psB", bufs=2, space="PSUM"))

            for stx in range(NB):
                load_x2(stx + 1)
                x2t = x2_tiles.pop(stx)
                ybf = [ybf_p.tile([128, C], BF16, tag="ybf", name="ybf")
                       for _ in range(2)]
                for tb in range(2):
                    _ln_tile(nc, pools, x2t[tb], ybf[tb],
                             (cb_t["g2b"], cb_t["be2b"]) if aff2 else None)
                ytT = []
                for cb in range(8):
                    pt = psB.tile([128, 256], BF16, tag="pt", name="pt")
                    for tb in range(2):
                        nc.tensor.transpose(
                            pt[:, tb * 128:(tb + 1) * 128],
                            ybf[tb][:, cb * 128:(cb + 1) * 128], idb[:])
                    t = yt_p.tile([128, 256], BF16, tag="yt", name="yt")
                    if cb % 2 == 0:
                        nc.vector.tensor_copy(t[:], pt[:])
                    else:
                        nc.scalar.copy(t[:], pt[:])
                    ytT.append(t)
                z2ps = [psB.tile([128, 512], F32, tag="acc", name="acc", bufs=4)
                        for _ in range(4)]
                for fb in range(32):
                    pz = psB.tile([128, 256], F32, tag="pz", name="pz")
                    for cb in range(8):
                        nc.tensor.matmul(pz[:],
                                         w1_sb[cb][:, fb * 128:(fb + 1) * 128],
                                         ytT[cb][:], start=(cb == 0), stop=(cb == 7))
                    z1 = z1_p.tile([128, 256], BF16, tag="z1", name="z1")
                    if use_b1:
                        nc.scalar.activation(z1[:], pz[:], AF.Relu,
                                             bias=b1t[:, fb:fb + 1])
                    else:
                        nc.scalar.activation(z1[:], pz[:], AF.Relu)
                    for tb in range(2):
                        for n in range(2):
                            for nh in range(2):
                                nc.tensor.matmul(
                                    z2ps[tb * 2 + n][:, nh * 256:(nh + 1) * 256],
                                    z1[:, tb * 128:(tb + 1) * 128],
                                    w2_sb[fb][:, n * 512 + nh * 256:
                                               n * 512 + (nh + 1) * 256],
                                    start=(fb == 0 and nh == 0),
                                    stop=(fb == 31 and nh == 1),
                                    skip_group_check=True)
                for tb in range(2):
                    u = u_p.tile([128, C], F32, tag="u", name="u")
                    for n in range(2):
                        nsl = slice(n * 512, (n + 1) * 512)
                        nc.vector.tensor_tensor(u[:, nsl], z2ps[tb * 2 + n][:],
                                                ybf[tb][:, nsl], ALU.add)
                    if use_b2:
                        nc.vector.tensor_tensor(u[:], u[:], cb_t["b2b"][:],
                                                ALU.add)
                    _ln_tile(nc, pools, u, u,
                             (cb_t["g3b"], cb_t["be3b"]) if aff3 else None)
                    nc.vector.tensor_tensor(x2t[tb][:], x2t[tb][:], u[:], ALU.add)
                    row = stx * 256 + tb * 128
                    nc.sync.dma_start(out=out_d[row:row + 128, :], in_=x2t[tb][:])
    nc.finalize()
    return nc


def _get_nc(flags):
    key = ("nc", flags)
    if key not in _CACHE:
        _CACHE[key] = _build(flags)
    return _CACHE[key]


def kernel(x, wq, wk, wv, w_proj, b_proj, w1, b1, w2, b2,
           g1, be1, g2, be2, g3, be3):
    bf = ml_dtypes.bfloat16
    x = np.asarray(x, np.float32)

    def nz(v):
        return bool(np.any(np.asarray(v, np.float32) != 0.0))

    def naff(g, be):
        return bool(np.any(np.asarray(g, np.float32) != 1.0)) or nz(be)

    flags = (naff(g1, be1), naff(g2, be2), naff(g3, be3),
             nz(b_proj), nz(b1), nz(b2))
    aff1, aff2, aff3, use_bproj, use_b1, use_b2 = flags
    nc = _get_nc(flags)

    def bc(vec):
        return np.ascontiguousarray(
            np.broadcast_to(np.asarray(vec, np.float32).reshape(1, C),
                            (128, C))).astype(bf)

    wqf = np.ascontiguousarray(
        np.asarray(wq, np.float32).transpose(1, 0, 2).reshape(C, C)).astype(bf)
    wkf = np.ascontiguousarray(
        np.asarray(wk, np.float32).transpose(1, 0, 2).reshape(C, C)).astype(bf)
    wvf = np.ascontiguousarray(
        np.asarray(wv, np.float32).transpose(1, 0, 2).reshape(C, C)).astype(bf)
    wpf = np.asarray(w_proj, np.float32).astype(bf)
    w1f = np.asarray(w1, np.float32).astype(bf)
    w2f = np.asarray(w2, np.float32).astype(bf)
    s = np.arange(128)[:, None]
    t = np.arange(128)[None, :]
    m0 = (s <= t).astype(np.float32).astype(bf)
    common = {
        "wqf": wqf, "wkf": wkf, "wvf": wvf, "wpf": wpf,
        "w1f": w1f, "w2f": w2f,
        "mask0": m0,
        "identb": np.eye(128, dtype=np.float32).astype(bf),
        "onesb": np.ones((128, 1), np.float32).astype(bf),
    }
    if use_b1:
        common["b1t"] = np.ascontiguousarray(
            np.asarray(b1, np.float32).reshape(F4 // 128, 128).T)
    if use_bproj:
        common["bprojb"] = bc(b_proj)
    if use_b2:
        common["b2b"] = bc(b2)
    if aff1:
        common["g1b"] = bc(g1)
        common["be1b"] = bc(be1)
    if aff2:
        common["g2b"] = bc(g2)
        common["be2b"] = bc(be2)
    if aff3:
        common["g3b"] = bc(g3)
        common["be3b"] = bc(be3)
    xs = x.reshape(NCORES, TOK, C)
    in_maps = [dict(common, x=np.ascontiguousarray(xs[i]))
               for i in range(NCORES)]
    import os
    trace = bool(os.environ.get("KERNEL_TRACE"))
    res = run_bass_kernel_spmd(nc, in_maps, core_ids=list(range(NCORES)),
                               trace=trace)
    _CACHE["last_res"] = res
    out = np.stack([res.results[i]["out"] for i in range(NCORES)], axis=0)
    return out.reshape(B, T, C).astype(np.float32)
